# revision 16
# baseline (speedup 1.0000x reference)
"""Causal self-attention (B=4, S=2048, D=1024, H=16) on 8 TRN2 NeuronCores.

Sharding: core = (batch b, head-group g) with b = core//2, g = core%2.
Each core computes, for its batch and its 8 heads: QKV projection,
transposed flash-style attention (scores kept k-major so the softmax'd
weights feed the PV matmul directly as the moving operand), and a partial
output projection over its heads' 512 hidden dims.  The host sums the two
partial projections per batch.

Numerics: matmul operands are fp16 (1 cycle/row on the PE vs 2 for fp32r,
accumulation still fp32 in PSUM); exp runs with a constant -8 shift so the
fp16 softmax weights can't overflow (the shift cancels in normalization).
End-to-end relative error ~1e-3 vs the fp32 reference.
"""

import numpy as np

B, S, D, H = 4, 2048, 1024, 16
HS = 64            # head size
NH = 8             # heads per core
C = 512            # per-core q/k/v width (NH * HS)
P = 128
NCORES = 8
DCH = D // P       # 8 contraction chunks for the projections
NW = S // 512      # 4 query windows of 512
NST = 4            # seq tiles (128 rows) per window
KCH = S // P       # 16 key chunks
ESHIFT = -4.0      # exp(0.25*s + ESHIFT); cancels in the softmax ratio

_cache = {}


def _build(has_bias=False):
    key = ("nc", has_bias)
    if key in _cache:
        return _cache[key]

    from contextlib import ExitStack

    import concourse.bass as bass
    import concourse.tile as tile
    from concourse import bacc, mybir
    from concourse.masks import make_identity, make_upper_triangular

    f32 = mybir.dt.float32
    f16 = mybir.dt.float16
    Exp = mybir.ActivationFunctionType.Exp
    Copy = mybir.ActivationFunctionType.Copy

    nc = bacc.Bacc(
        "TRN2", target_bir_lowering=False, debug=False, num_devices=NCORES
    )

    x_d = nc.dram_tensor("x", [S, D], f16, kind="ExternalInput").ap()
    wq_d = nc.dram_tensor("wq", [D, C], f16, kind="ExternalInput").ap()
    wk_d = nc.dram_tensor("wk", [D, C], f16, kind="ExternalInput").ap()
    wv_d = nc.dram_tensor("wv", [D, C], f16, kind="ExternalInput").ap()
    wp_d = nc.dram_tensor("wp", [C, D], f16, kind="ExternalInput").ap()
    # per-partition bias columns: col j<4 -> q col-tile j, col 4+j -> k col-tile j
    bqk_d = nc.dram_tensor("bqk", [P, 8], f32, kind="ExternalInput").ap()
    bv_d = nc.dram_tensor("bv", [1, C], f32, kind="ExternalInput").ap()
    bp_d = nc.dram_tensor("bp", [1, D], f32, kind="ExternalInput").ap()
    out_d = nc.dram_tensor("out", [S, D], f32, kind="ExternalOutput").ap()

    with tile.TileContext(nc) as tc, ExitStack() as ctx:
        ctx.enter_context(nc.allow_low_precision(reason="fp16 attention"))

        const = ctx.enter_context(tc.tile_pool(name="const", bufs=1))
        persist = ctx.enter_context(tc.tile_pool(name="persist", bufs=1))
        xload = ctx.enter_context(tc.tile_pool(name="xload", bufs=3))
        xtp = ctx.enter_context(tc.tile_pool(name="xtp", bufs=2))
        qtw = ctx.enter_context(tc.tile_pool(name="qtw", bufs=2))
        otw = ctx.enter_context(tc.tile_pool(name="otw", bufs=2))
        expool = ctx.enter_context(tc.tile_pool(name="expool", bufs=4))
        denpool = ctx.enter_context(tc.tile_pool(name="denpool", bufs=3))
        rhpool = ctx.enter_context(tc.tile_pool(name="rhpool", bufs=2))
        bcpool = ctx.enter_context(tc.tile_pool(name="bcpool", bufs=3))
        stpool = ctx.enter_context(tc.tile_pool(name="stpool", bufs=3))

        pS = ctx.enter_context(tc.tile_pool(name="pS", bufs=2, space="PSUM"))
        pPV = ctx.enter_context(tc.tile_pool(name="pPV", bufs=1, space="PSUM"))
        pDEN = ctx.enter_context(tc.tile_pool(name="pDEN", bufs=1, space="PSUM"))
        pMISC = ctx.enter_context(tc.tile_pool(name="pMISC", bufs=1, space="PSUM"))
        pAUX = ctx.enter_context(tc.tile_pool(name="pAUX", bufs=1, space="PSUM"))

        identf = const.tile([P, P], f32, tag="identf")
        make_identity(nc, identf)
        ident = const.tile([P, P], f16, tag="ident")
        nc.vector.tensor_copy(ident, identf)
        tri = const.tile([P, P], f32, tag="tri")
        make_upper_triangular(nc, tri, val=1.0, diag=True)  # tri[k,q]=1 iff q>=k
        zeros = const.tile([P, 384], f32, tag="zeros")
        nc.vector.memset(zeros, 0.0)
        eshift_sb = const.tile([P, 1], f32, tag="eshift")
        nc.vector.memset(eshift_sb, ESHIFT)
        ones8 = const.tile([P, 8], f32, tag="ones8")
        nc.vector.memset(ones8, 1.0)
        ohf = const.tile([1, 64], f32, tag="ohf")
        nc.vector.memset(ohf, 0.0)
        for h in range(NH):
            nc.vector.memset(ohf[0:1, h * 9:h * 9 + 1], 1.0)
        onehot = const.tile([1, NH, NH], f16, tag="onehot")
        nc.vector.tensor_copy(onehot, ohf[:].rearrange("p (a b) -> p a b", b=NH))
        bandf = const.tile([NH, 512], f32, tag="bandf")
        nc.gpsimd.memset(bandf, 1.0)
        nc.gpsimd.affine_select(
            out=bandf, in_=bandf, compare_op=mybir.AluOpType.is_ge,
            fill=0.0, base=0, pattern=[[1, 512]], channel_multiplier=-64)
        nc.gpsimd.affine_select(
            out=bandf, in_=bandf, compare_op=mybir.AluOpType.is_ge,
            fill=0.0, base=63, pattern=[[-1, 512]], channel_multiplier=64)
        sel = const.tile([NH, 512], f16, tag="sel")
        nc.vector.tensor_copy(sel, bandf[:])

        bqk_sb = const.tile([P, 8], f32, tag="bqk")
        nc.sync.dma_start(bqk_sb, bqk_d)
        bv_bc = const.tile([P, C], f32, tag="bv_bc")
        nc.sync.dma_start(
            bv_bc,
            bass.AP(tensor=bv_d.tensor, offset=bv_d.offset,
                    ap=[[0, P], list(bv_d.ap[-1])]),
        )
        bp_bc = const.tile([P, D], f32, tag="bp_bc")
        nc.sync.dma_start(
            bp_bc,
            bass.AP(tensor=bp_d.tensor, offset=bp_d.offset,
                    ap=[[0, P], list(bp_d.ap[-1])]),
        )

        wq_sb = persist.tile([P, DCH, C], f16, tag="wq")
        wk_sb = persist.tile([P, DCH, C], f16, tag="wk")
        wv_sb = persist.tile([P, DCH, C], f16, tag="wv")
        wp_sb = persist.tile([P, 4, D], f16, tag="wp")
        KT = persist.tile([P, 4, S], f16, tag="KT")
        Vt = persist.tile([P, KCH, NH * 65], f16, tag="Vt")

        # ---------- unit generators (emitted lazily for interleaving) ----------

        def xT_units(w, xT):
            """Load x rows of window w and transpose into xT [P, DCH, 512]."""
            for st in range(NST):
                def unit(w=w, st=st, xT=xT):
                    xt = xload.tile([P, D], f16, tag="xt")
                    row0 = (4 * w + st) * P
                    nc.sync.dma_start(xt, x_d[row0:row0 + P, :])
                    for dh in range(2):
                        ptr = pAUX.tile([P, 4, P], f16, tag="aux")
                        for j in range(4):
                            d = 4 * dh + j
                            nc.tensor.transpose(
                                ptr[:, j, :], xt[:, d * P:(d + 1) * P], ident[:])
                        nc.vector.tensor_copy(
                            xT[:, 4 * dh:4 * dh + 4, st * P:(st + 1) * P], ptr[:])
                yield unit

        def qk_units(w, xT, qt):
            """Q^T / K^T projections for window w from xT."""
            for ct in range(4):
                for qk in range(2):
                    def unit(w=w, ct=ct, qk=qk, xT=xT, qt=qt):
                        ps = pAUX.tile([P, 512], f32, tag="aux")
                        wsb = wq_sb if qk == 0 else wk_sb
                        for d in range(DCH):
                            nc.tensor.matmul(
                                ps, wsb[:, d, ct * P:(ct + 1) * P], xT[:, d, :],
                                start=(d == 0), stop=(d == DCH - 1))
                        dest = (qt[:, ct, :] if qk == 0
                                else KT[:, ct, w * 512:(w + 1) * 512])
                        if has_bias:
                            nc.vector.tensor_scalar_add(
                                dest, ps,
                                bqk_sb[:, qk * 4 + ct:qk * 4 + ct + 1])
                        else:
                            nc.vector.tensor_copy(dest, ps)
                    yield unit

        def v_units(w, xT):
            """V for the 4 key chunks of window w, head-grouped with ones col."""
            for st in range(NST):
                def unit(w=w, st=st, xT=xT):
                    kc = 4 * w + st
                    ps = pAUX.tile([P, 512], f32, tag="aux")
                    for d in range(DCH):
                        nc.tensor.matmul(ps, xT[:, d, st * P:(st + 1) * P],
                                         wv_sb[:, d, :],
                                         start=(d == 0), stop=(d == DCH - 1))
                    vtv = Vt[:, kc, :].rearrange("p (h c) -> p h c", c=65)
                    if has_bias:
                        nc.vector.tensor_add(
                            vtv[:, :, 0:64],
                            ps.rearrange("p (h c) -> p h c", c=64),
                            bv_bc[:].rearrange("p (h c) -> p h c", c=64))
                    else:
                        nc.vector.tensor_copy(
                            vtv[:, :, 0:64],
                            ps.rearrange("p (h c) -> p h c", c=64))
                    nc.vector.tensor_copy(vtv[:, :, 64:65],
                                          ones8[:, :].unsqueeze(2))
                yield unit

        def proj_units(w, ot):
            """Output projection for window w's 512 seq rows (partial over C)."""
            for st in range(NST):
                for cw in range(2):
                    def unit(w=w, st=st, cw=cw, ot=ot):
                        pp = pMISC.tile([P, 512], f32, tag="misc")
                        for j in range(4):
                            nc.tensor.matmul(
                                pp, ot[:, j, st * P:(st + 1) * P],
                                wp_sb[:, j, cw * 512:(cw + 1) * 512],
                                start=(j == 0), stop=(j == 3))
                        stg = stpool.tile([P, 512], f32, tag="stg")
                        if has_bias:
                            nc.vector.tensor_add(
                                stg, pp, bp_bc[:, cw * 512:(cw + 1) * 512])
                        else:
                            nc.vector.tensor_copy(stg, pp)
                        row0 = (4 * w + st) * P
                        nc.sync.dma_start(
                            out_d[row0:row0 + P, cw * 512:(cw + 1) * 512], stg)
                    yield unit

        def attention_head(w, h, qt, ot, den_ps):
            """Scores+exp+PV for one head; unnormalized O -> ot, denominator
            scattered into row h of den_ps via a K=1 one-hot matmul."""
            ct, po = h // 2, (h % 2) * 64
            pv = pPV.tile([65, 512], f32, tag="pv")
            last_kc = 4 * w + 3
            for g2 in range(2 * (w + 1)):      # 2-chunk half groups
                sc = pS.tile([P, 2, 512], f32, tag="sc")
                for rr in range(2):
                    kc = 2 * g2 + rr
                    nc.tensor.matmul(
                        sc[:, rr, :],
                        KT[po:po + 64, ct, kc * P:(kc + 1) * P],
                        qt[po:po + 64, ct, :],
                        start=True, stop=True)
                ex = expool.tile([P, 2, 512], f16, tag="ex")
                nc.scalar.activation(ex, sc, Exp, scale=0.25, bias=eshift_sb[:])
                for rr in range(2):
                    kc = 2 * g2 + rr
                    r = kc - 4 * w            # >=0 only inside the diag block
                    q0 = 0
                    if r >= 0:
                        # only q >= r*P can attend this chunk; clip the PV
                        # extent instead of zero-filling the masked region
                        q0 = r * P
                        nc.vector.tensor_mul(ex[:, rr, q0:q0 + P],
                                             ex[:, rr, q0:q0 + P], tri[:])
                    nc.tensor.matmul(pv[:, q0:], Vt[:, kc, h * 65:(h + 1) * 65],
                                     ex[:, rr, q0:],
                                     start=(kc == 0), stop=(kc == last_kc))
            # stash unnormalized O scaled by 1/4 (fp16 range headroom);
            # scatter the denominator into den_ps[h]
            nc.vector.tensor_scalar_mul(ot[po:po + 64, ct, :], pv[0:64, :], 0.25)
            den_h = denpool.tile([1, 512], f16, tag="den")
            nc.vector.tensor_copy(den_h, pv[64:65, :])
            nc.tensor.matmul(den_ps, onehot[:, h, :], den_h[:],
                             start=(h == 0), stop=(h == NH - 1))

        def norm_units(w, ot, den_ps):
            """Window-batched softmax normalization of ot (in place)."""
            shared = {}

            def recip_unit(ot=ot, den_ps=den_ps):
                rall = rhpool.tile([NH, 512], f32, tag="rall")
                nc.vector.reciprocal(rall, den_ps[:])
                rh = rhpool.tile([NH, 512], f16, tag="rh")
                # x4 so small reciprocals clear the fp16 subnormal floor; the
                # 4*(1/4) pair cancels in the final normalize multiply
                nc.vector.tensor_scalar_mul(rh, rall, 4.0)
                shared["rh"] = rh
            yield recip_unit
            for j in range(4):      # one unit per column-tile = 2 heads
                def unit(j=j, ot=ot):
                    rh = shared["rh"]
                    # sel[:, 128j:128j+128] row m selects head 2j + (m>=64)
                    bcp = pMISC.tile([P, 512], f32, tag="misc")
                    nc.tensor.matmul(bcp, sel[:, j * P:(j + 1) * P],
                                     rh[:], start=True, stop=True)
                    bc_sb = bcpool.tile([P, 512], f32, tag="bc")
                    nc.any.tensor_copy(out=bc_sb, in_=bcp)
                    nc.vector.tensor_mul(ot[:, j, :], ot[:, j, :], bc_sb[:])
                yield unit

        # ---------------------------- schedule ----------------------------

        xT_cur = xtp.tile([P, DCH, 512], f16, tag="xT")
        qt_cur = qtw.tile([P, 4, 512], f16, tag="qt")
        xu0 = list(xT_units(0, xT_cur))
        xu0[0]()                      # x tile 0 in flight first: PE starts early
        for d in range(DCH):          # then stream the projection weights
            nc.sync.dma_start(wq_sb[:, d, :], wq_d[d * P:(d + 1) * P, :])
            nc.sync.dma_start(wk_sb[:, d, :], wk_d[d * P:(d + 1) * P, :])
        for u in xu0[1:]:
            u()
        for d in range(DCH):
            nc.sync.dma_start(wv_sb[:, d, :], wv_d[d * P:(d + 1) * P, :])
        for u in qk_units(0, xT_cur, qt_cur):
            u()
        for u in v_units(0, xT_cur):
            u()
        for j in range(4):            # proj weights are not needed until w1
            nc.sync.dma_start(wp_sb[:, j, :], wp_d[j * P:(j + 1) * P, :])

        pending = []            # units that must run after window w's heads
        for w in range(NW):
            ot_cur = otw.tile([P, 4, 512], f16, tag="ot")
            den_ps = pDEN.tile([NH, 512], f32, tag="den_ps")

            bg = list(pending)
            pending = []
            if w + 1 < NW:
                xT_next = xtp.tile([P, DCH, 512], f16, tag="xT")
                qt_next = qtw.tile([P, 4, 512], f16, tag="qt")
                bg += list(xT_units(w + 1, xT_next))
                bg += list(qk_units(w + 1, xT_next, qt_next))
                bg += list(v_units(w + 1, xT_next))
            else:
                xT_next = qt_next = None

            for h in range(NH):
                attention_head(w, h, qt_cur, ot_cur, den_ps)
                n_take = (len(bg) * (h + 1)) // NH - (len(bg) * h) // NH
                for _ in range(n_take):
                    bg.pop(0)()
            for u in bg:
                u()

            # reciprocal runs right away (den_ps is complete; frees its psum
            # before the next window's scatter); bc+proj interleave into the
            # next window's attention
            units = list(norm_units(w, ot_cur, den_ps))
            units[0]()
            pending = units[1:] + list(proj_units(w, ot_cur))

            xT_cur, qt_cur = xT_next, qt_next

        for u in pending:
            u()

    nc.compile()
    _cache[key] = nc
    return nc


def _make_in_maps(input_data, w_qkv, b_qkv, w_proj, b_proj):
    x = np.asarray(input_data, dtype=np.float32).astype(np.float16)
    wqkv = np.asarray(w_qkv, dtype=np.float32).astype(np.float16)
    bqkv = np.asarray(b_qkv, dtype=np.float32)
    wp = np.asarray(w_proj, dtype=np.float32).astype(np.float16)
    bp = np.asarray(b_proj, dtype=np.float32)

    in_maps = []
    for core in range(NCORES):
        b, g = core // 2, core % 2
        cs = slice(g * C, (g + 1) * C)
        bq = bqkv[0 * D:1 * D][cs]
        bk = bqkv[1 * D:2 * D][cs]
        bqk = np.empty((P, 8), np.float32)
        for j in range(4):
            bqk[:, j] = bq[j * P:(j + 1) * P]
            bqk[:, 4 + j] = bk[j * P:(j + 1) * P]
        in_maps.append({
            "x": np.ascontiguousarray(x[b]),
            "wq": np.ascontiguousarray(wqkv[:, 0 * D:1 * D][:, cs]),
            "wk": np.ascontiguousarray(wqkv[:, 1 * D:2 * D][:, cs]),
            "wv": np.ascontiguousarray(wqkv[:, 2 * D:3 * D][:, cs]),
            "wp": np.ascontiguousarray(wp[cs, :]),
            "bqk": bqk,
            "bv": np.ascontiguousarray(bqkv[2 * D:3 * D][cs]).reshape(1, C),
            "bp": (bp if g == 0 else np.zeros_like(bp)).reshape(1, D),
        })
    return in_maps


def kernel(input_data, w_qkv, b_qkv, w_proj, b_proj):
    from concourse.bass_utils import run_bass_kernel_spmd

    nc = _build()
    in_maps = _make_in_maps(input_data, w_qkv, b_qkv, w_proj, b_proj)
    res = run_bass_kernel_spmd(nc, in_maps, core_ids=list(range(NCORES)))
    parts = [np.asarray(res.results[i]["out"]) for i in range(NCORES)]
    out = np.stack([parts[2 * b] + parts[2 * b + 1] for b in range(B)])
    return out.astype(np.float32)


# revision 17
# speedup vs baseline: 1.0152x; 1.0152x over previous
"""Causal self-attention (B=4, S=2048, D=1024, H=16) on 8 TRN2 NeuronCores.

Sharding: core = (batch b, head-group g) with b = core//2, g = core%2.
Each core computes, for its batch and its 8 heads: QKV projection,
transposed flash-style attention (scores kept k-major so the softmax'd
weights feed the PV matmul directly as the moving operand), and a partial
output projection over its heads' 512 hidden dims.  The host sums the two
partial projections per batch.

Numerics: matmul operands are fp16 (1 cycle/row on the PE vs 2 for fp32r,
accumulation still fp32 in PSUM); exp runs with a constant -8 shift so the
fp16 softmax weights can't overflow (the shift cancels in normalization).
End-to-end relative error ~1e-3 vs the fp32 reference.
"""

import numpy as np

B, S, D, H = 4, 2048, 1024, 16
HS = 64            # head size
NH = 8             # heads per core
C = 512            # per-core q/k/v width (NH * HS)
P = 128
NCORES = 8
DCH = D // P       # 8 contraction chunks for the projections
NW = S // 512      # 4 query windows of 512
NST = 4            # seq tiles (128 rows) per window
KCH = S // P       # 16 key chunks
ESHIFT = -4.0      # exp(0.25*s + ESHIFT); cancels in the softmax ratio

_cache = {}


def _build(has_bias=False):
    key = ("nc", has_bias)
    if key in _cache:
        return _cache[key]

    from contextlib import ExitStack

    import concourse.bass as bass
    import concourse.tile as tile
    from concourse import bacc, mybir
    from concourse.masks import make_identity, make_upper_triangular

    f32 = mybir.dt.float32
    f16 = mybir.dt.float16
    Exp = mybir.ActivationFunctionType.Exp
    Copy = mybir.ActivationFunctionType.Copy

    nc = bacc.Bacc(
        "TRN2", target_bir_lowering=False, debug=False, num_devices=NCORES
    )

    x_d = nc.dram_tensor("x", [S, D], f16, kind="ExternalInput").ap()
    wq_d = nc.dram_tensor("wq", [D, C], f16, kind="ExternalInput").ap()
    wk_d = nc.dram_tensor("wk", [D, C], f16, kind="ExternalInput").ap()
    wv_d = nc.dram_tensor("wv", [D, C], f16, kind="ExternalInput").ap()
    wp_d = nc.dram_tensor("wp", [C, D], f16, kind="ExternalInput").ap()
    # per-partition bias columns: col j<4 -> q col-tile j, col 4+j -> k col-tile j
    bqk_d = nc.dram_tensor("bqk", [P, 8], f32, kind="ExternalInput").ap()
    bv_d = nc.dram_tensor("bv", [1, C], f32, kind="ExternalInput").ap()
    bp_d = nc.dram_tensor("bp", [1, D], f32, kind="ExternalInput").ap()
    out_d = nc.dram_tensor("out", [S, D], f32, kind="ExternalOutput").ap()

    with tile.TileContext(nc) as tc, ExitStack() as ctx:
        ctx.enter_context(nc.allow_low_precision(reason="fp16 attention"))

        const = ctx.enter_context(tc.tile_pool(name="const", bufs=1))
        persist = ctx.enter_context(tc.tile_pool(name="persist", bufs=1))
        xload = ctx.enter_context(tc.tile_pool(name="xload", bufs=3))
        xtp = ctx.enter_context(tc.tile_pool(name="xtp", bufs=2))
        qtw = ctx.enter_context(tc.tile_pool(name="qtw", bufs=2))
        otw = ctx.enter_context(tc.tile_pool(name="otw", bufs=2))
        expool = ctx.enter_context(tc.tile_pool(name="expool", bufs=4))
        denpool = ctx.enter_context(tc.tile_pool(name="denpool", bufs=3))
        rhpool = ctx.enter_context(tc.tile_pool(name="rhpool", bufs=2))
        bcpool = ctx.enter_context(tc.tile_pool(name="bcpool", bufs=3))
        stpool = ctx.enter_context(tc.tile_pool(name="stpool", bufs=3))

        pS = ctx.enter_context(tc.tile_pool(name="pS", bufs=2, space="PSUM"))
        pPV = ctx.enter_context(tc.tile_pool(name="pPV", bufs=1, space="PSUM"))
        pDEN = ctx.enter_context(tc.tile_pool(name="pDEN", bufs=1, space="PSUM"))
        pMISC = ctx.enter_context(tc.tile_pool(name="pMISC", bufs=1, space="PSUM"))
        pAUX = ctx.enter_context(tc.tile_pool(name="pAUX", bufs=1, space="PSUM"))

        identf = const.tile([P, P], f32, tag="identf")
        make_identity(nc, identf)
        ident = const.tile([P, P], f16, tag="ident")
        nc.vector.tensor_copy(ident, identf)
        tri = const.tile([P, P], f32, tag="tri")
        make_upper_triangular(nc, tri, val=1.0, diag=True)  # tri[k,q]=1 iff q>=k
        zeros = const.tile([P, 384], f32, tag="zeros")
        nc.vector.memset(zeros, 0.0)
        eshift_sb = const.tile([P, 1], f32, tag="eshift")
        nc.vector.memset(eshift_sb, ESHIFT)
        ones8 = const.tile([P, 8], f32, tag="ones8")
        nc.vector.memset(ones8, 1.0)
        ohf = const.tile([1, 64], f32, tag="ohf")
        nc.vector.memset(ohf, 0.0)
        for h in range(NH):
            nc.vector.memset(ohf[0:1, h * 9:h * 9 + 1], 1.0)
        onehot = const.tile([1, NH, NH], f16, tag="onehot")
        nc.vector.tensor_copy(onehot, ohf[:].rearrange("p (a b) -> p a b", b=NH))
        bandf = const.tile([NH, 512], f32, tag="bandf")
        nc.gpsimd.memset(bandf, 1.0)
        nc.gpsimd.affine_select(
            out=bandf, in_=bandf, compare_op=mybir.AluOpType.is_ge,
            fill=0.0, base=0, pattern=[[1, 512]], channel_multiplier=-64)
        nc.gpsimd.affine_select(
            out=bandf, in_=bandf, compare_op=mybir.AluOpType.is_ge,
            fill=0.0, base=63, pattern=[[-1, 512]], channel_multiplier=64)
        sel = const.tile([NH, 512], f16, tag="sel")
        nc.vector.tensor_copy(sel, bandf[:])

        bqk_sb = const.tile([P, 8], f32, tag="bqk")
        nc.sync.dma_start(bqk_sb, bqk_d)
        bv_bc = const.tile([P, C], f32, tag="bv_bc")
        nc.sync.dma_start(
            bv_bc,
            bass.AP(tensor=bv_d.tensor, offset=bv_d.offset,
                    ap=[[0, P], list(bv_d.ap[-1])]),
        )
        bp_bc = const.tile([P, D], f32, tag="bp_bc")
        nc.sync.dma_start(
            bp_bc,
            bass.AP(tensor=bp_d.tensor, offset=bp_d.offset,
                    ap=[[0, P], list(bp_d.ap[-1])]),
        )

        wq_sb = persist.tile([P, DCH, C], f16, tag="wq")
        wk_sb = persist.tile([P, DCH, C], f16, tag="wk")
        wv_sb = persist.tile([P, DCH, C], f16, tag="wv")
        wp_sb = persist.tile([P, 4, D], f16, tag="wp")
        KT = persist.tile([P, 4, S], f16, tag="KT")
        Vt = persist.tile([P, KCH, NH * 65], f16, tag="Vt")

        # ---------- unit generators (emitted lazily for interleaving) ----------

        def xT_units(w, xT):
            """Load x rows of window w and transpose into xT [P, DCH, 512]."""
            for st in range(NST):
                def unit(w=w, st=st, xT=xT):
                    xt = xload.tile([P, D], f16, tag="xt")
                    row0 = (4 * w + st) * P
                    nc.sync.dma_start(xt, x_d[row0:row0 + P, :])
                    for dh in range(2):
                        ptr = pAUX.tile([P, 4, P], f16, tag="aux")
                        for j in range(4):
                            d = 4 * dh + j
                            nc.tensor.transpose(
                                ptr[:, j, :], xt[:, d * P:(d + 1) * P], ident[:])
                        nc.vector.tensor_copy(
                            xT[:, 4 * dh:4 * dh + 4, st * P:(st + 1) * P], ptr[:])
                yield unit

        def qk_units(w, xT, qt):
            """Q^T / K^T projections for window w from xT."""
            for ct in range(4):
                for qk in range(2):
                    def unit(w=w, ct=ct, qk=qk, xT=xT, qt=qt):
                        ps = pAUX.tile([P, 512], f32, tag="aux")
                        wsb = wq_sb if qk == 0 else wk_sb
                        for d in range(DCH):
                            nc.tensor.matmul(
                                ps, wsb[:, d, ct * P:(ct + 1) * P], xT[:, d, :],
                                start=(d == 0), stop=(d == DCH - 1))
                        dest = (qt[:, ct, :] if qk == 0
                                else KT[:, ct, w * 512:(w + 1) * 512])
                        if has_bias:
                            nc.vector.tensor_scalar_add(
                                dest, ps,
                                bqk_sb[:, qk * 4 + ct:qk * 4 + ct + 1])
                        else:
                            nc.vector.tensor_copy(dest, ps)
                    yield unit

        def v_units(w, xT):
            """V for the 4 key chunks of window w, head-grouped with ones col."""
            for st in range(NST):
                def unit(w=w, st=st, xT=xT):
                    kc = 4 * w + st
                    ps = pAUX.tile([P, 512], f32, tag="aux")
                    for d in range(DCH):
                        nc.tensor.matmul(ps, xT[:, d, st * P:(st + 1) * P],
                                         wv_sb[:, d, :],
                                         start=(d == 0), stop=(d == DCH - 1))
                    vtv = Vt[:, kc, :].rearrange("p (h c) -> p h c", c=65)
                    if has_bias:
                        nc.vector.tensor_add(
                            vtv[:, :, 0:64],
                            ps.rearrange("p (h c) -> p h c", c=64),
                            bv_bc[:].rearrange("p (h c) -> p h c", c=64))
                    else:
                        nc.vector.tensor_copy(
                            vtv[:, :, 0:64],
                            ps.rearrange("p (h c) -> p h c", c=64))
                    nc.vector.tensor_copy(vtv[:, :, 64:65],
                                          ones8[:, :].unsqueeze(2))
                yield unit

        def proj_units(w, ot):
            """Output projection for window w's 512 seq rows (partial over C)."""
            for st in range(NST):
                for cw in range(2):
                    def unit(w=w, st=st, cw=cw, ot=ot):
                        pp = pMISC.tile([P, 512], f32, tag="misc")
                        for j in range(4):
                            nc.tensor.matmul(
                                pp, ot[:, j, st * P:(st + 1) * P],
                                wp_sb[:, j, cw * 512:(cw + 1) * 512],
                                start=(j == 0), stop=(j == 3))
                        stg = stpool.tile([P, 512], f32, tag="stg")
                        if has_bias:
                            nc.vector.tensor_add(
                                stg, pp, bp_bc[:, cw * 512:(cw + 1) * 512])
                        else:
                            nc.vector.tensor_copy(stg, pp)
                        row0 = (4 * w + st) * P
                        nc.sync.dma_start(
                            out_d[row0:row0 + P, cw * 512:(cw + 1) * 512], stg)
                    yield unit

        def attention_head(w, h, qt, ot, den_ps):
            """Scores+exp+PV for one head; unnormalized O -> ot, denominator
            scattered into row h of den_ps via a K=1 one-hot matmul."""
            ct, po = h // 2, (h % 2) * 64
            pv = pPV.tile([65, 512], f32, tag="pv")
            last_kc = 4 * w + 3
            for g2 in range(2 * (w + 1)):      # 2-chunk half groups
                sc = pS.tile([P, 2, 512], f32, tag="sc")
                for rr in range(2):
                    kc = 2 * g2 + rr
                    nc.tensor.matmul(
                        sc[:, rr, :],
                        KT[po:po + 64, ct, kc * P:(kc + 1) * P],
                        qt[po:po + 64, ct, :],
                        start=True, stop=True)
                ex = expool.tile([P, 2, 512], f16, tag="ex")
                nc.scalar.activation(ex, sc, Exp, scale=0.25, bias=eshift_sb[:])
                for rr in range(2):
                    kc = 2 * g2 + rr
                    r = kc - 4 * w            # >=0 only inside the diag block
                    q0 = 0
                    if r >= 0:
                        # only q >= r*P can attend this chunk; clip the PV
                        # extent instead of zero-filling the masked region
                        q0 = r * P
                        nc.vector.tensor_mul(ex[:, rr, q0:q0 + P],
                                             ex[:, rr, q0:q0 + P], tri[:])
                    nc.tensor.matmul(pv[:, q0:], Vt[:, kc, h * 65:(h + 1) * 65],
                                     ex[:, rr, q0:],
                                     start=(kc == 0), stop=(kc == last_kc))
            # stash unnormalized O scaled by 1/4 (fp16 range headroom);
            # scatter the denominator into den_ps[h]
            nc.vector.tensor_scalar_mul(ot[po:po + 64, ct, :], pv[0:64, :], 0.25)
            den_h = denpool.tile([1, 512], f16, tag="den")
            nc.vector.tensor_copy(den_h, pv[64:65, :])
            nc.tensor.matmul(den_ps, onehot[:, h, :], den_h[:],
                             start=(h == 0), stop=(h == NH - 1))

        def norm_units(w, ot, den_ps):
            """Window-batched softmax normalization of ot (in place)."""
            shared = {}

            def recip_unit(ot=ot, den_ps=den_ps):
                rall = rhpool.tile([NH, 512], f32, tag="rall")
                nc.vector.reciprocal(rall, den_ps[:])
                rh = rhpool.tile([NH, 512], f16, tag="rh")
                # x4 so small reciprocals clear the fp16 subnormal floor; the
                # 4*(1/4) pair cancels in the final normalize multiply
                nc.vector.tensor_scalar_mul(rh, rall, 4.0)
                shared["rh"] = rh
            yield recip_unit
            for j in range(4):      # one unit per column-tile = 2 heads
                def unit(j=j, ot=ot):
                    rh = shared["rh"]
                    # sel[:, 128j:128j+128] row m selects head 2j + (m>=64)
                    bcp = pMISC.tile([P, 512], f32, tag="misc")
                    nc.tensor.matmul(bcp, sel[:, j * P:(j + 1) * P],
                                     rh[:], start=True, stop=True)
                    bc_sb = bcpool.tile([P, 512], f32, tag="bc")
                    nc.scalar.activation(bc_sb, bcp, Copy)
                    nc.vector.tensor_mul(ot[:, j, :], ot[:, j, :], bc_sb[:])
                yield unit

        # ---------------------------- schedule ----------------------------

        xT_cur = xtp.tile([P, DCH, 512], f16, tag="xT")
        qt_cur = qtw.tile([P, 4, 512], f16, tag="qt")
        xu0 = list(xT_units(0, xT_cur))
        xu0[0]()                      # x tile 0 in flight first: PE starts early
        for d in range(DCH):          # then stream the projection weights
            nc.sync.dma_start(wq_sb[:, d, :], wq_d[d * P:(d + 1) * P, :])
            nc.sync.dma_start(wk_sb[:, d, :], wk_d[d * P:(d + 1) * P, :])
        for u in xu0[1:]:
            u()
        for d in range(DCH):
            nc.sync.dma_start(wv_sb[:, d, :], wv_d[d * P:(d + 1) * P, :])
        for u in qk_units(0, xT_cur, qt_cur):
            u()
        for u in v_units(0, xT_cur):
            u()
        for j in range(4):            # proj weights are not needed until w1
            nc.sync.dma_start(wp_sb[:, j, :], wp_d[j * P:(j + 1) * P, :])

        pending = []            # units that must run after window w's heads
        for w in range(NW):
            ot_cur = otw.tile([P, 4, 512], f16, tag="ot")
            den_ps = pDEN.tile([NH, 512], f32, tag="den_ps")

            bg = list(pending)
            pending = []
            if w + 1 < NW:
                xT_next = xtp.tile([P, DCH, 512], f16, tag="xT")
                qt_next = qtw.tile([P, 4, 512], f16, tag="qt")
                bg += list(xT_units(w + 1, xT_next))
                bg += list(qk_units(w + 1, xT_next, qt_next))
                bg += list(v_units(w + 1, xT_next))
            else:
                xT_next = qt_next = None

            for h in range(NH):
                attention_head(w, h, qt_cur, ot_cur, den_ps)
                n_take = (len(bg) * (h + 1)) // NH - (len(bg) * h) // NH
                for _ in range(n_take):
                    bg.pop(0)()
            for u in bg:
                u()

            # reciprocal runs right away (den_ps is complete; frees its psum
            # before the next window's scatter); bc+proj interleave into the
            # next window's attention
            units = list(norm_units(w, ot_cur, den_ps))
            units[0]()
            pending = units[1:] + list(proj_units(w, ot_cur))

            xT_cur, qt_cur = xT_next, qt_next

        for u in pending:
            u()

    nc.compile()
    _cache[key] = nc
    return nc


def _make_in_maps(input_data, w_qkv, b_qkv, w_proj, b_proj):
    x = np.asarray(input_data, dtype=np.float32).astype(np.float16)
    wqkv = np.asarray(w_qkv, dtype=np.float32).astype(np.float16)
    bqkv = np.asarray(b_qkv, dtype=np.float32)
    wp = np.asarray(w_proj, dtype=np.float32).astype(np.float16)
    bp = np.asarray(b_proj, dtype=np.float32)

    in_maps = []
    for core in range(NCORES):
        b, g = core // 2, core % 2
        cs = slice(g * C, (g + 1) * C)
        bq = bqkv[0 * D:1 * D][cs]
        bk = bqkv[1 * D:2 * D][cs]
        bqk = np.empty((P, 8), np.float32)
        for j in range(4):
            bqk[:, j] = bq[j * P:(j + 1) * P]
            bqk[:, 4 + j] = bk[j * P:(j + 1) * P]
        in_maps.append({
            "x": np.ascontiguousarray(x[b]),
            "wq": np.ascontiguousarray(wqkv[:, 0 * D:1 * D][:, cs]),
            "wk": np.ascontiguousarray(wqkv[:, 1 * D:2 * D][:, cs]),
            "wv": np.ascontiguousarray(wqkv[:, 2 * D:3 * D][:, cs]),
            "wp": np.ascontiguousarray(wp[cs, :]),
            "bqk": bqk,
            "bv": np.ascontiguousarray(bqkv[2 * D:3 * D][cs]).reshape(1, C),
            "bp": (bp if g == 0 else np.zeros_like(bp)).reshape(1, D),
        })
    return in_maps


def kernel(input_data, w_qkv, b_qkv, w_proj, b_proj):
    from concourse.bass_utils import run_bass_kernel_spmd

    nc = _build()
    in_maps = _make_in_maps(input_data, w_qkv, b_qkv, w_proj, b_proj)
    res = run_bass_kernel_spmd(nc, in_maps, core_ids=list(range(NCORES)))
    parts = [np.asarray(res.results[i]["out"]) for i in range(NCORES)]
    out = np.stack([parts[2 * b] + parts[2 * b + 1] for b in range(B)])
    return out.astype(np.float32)


# revision 18
# speedup vs baseline: 1.0338x; 1.0183x over previous
"""Causal self-attention (B=4, S=2048, D=1024, H=16) on 8 TRN2 NeuronCores.

Sharding: core = (batch b, head-group g) with b = core//2, g = core%2.
Each core computes, for its batch and its 8 heads: QKV projection,
transposed flash-style attention (scores kept k-major so the softmax'd
weights feed the PV matmul directly as the moving operand), and a partial
output projection over its heads' 512 hidden dims.  The host sums the two
partial projections per batch.

Numerics: matmul operands are fp16 (1 cycle/row on the PE vs 2 for fp32r,
accumulation still fp32 in PSUM); exp runs with a constant -8 shift so the
fp16 softmax weights can't overflow (the shift cancels in normalization).
End-to-end relative error ~1e-3 vs the fp32 reference.
"""

import numpy as np

B, S, D, H = 4, 2048, 1024, 16
HS = 64            # head size
NH = 8             # heads per core
C = 512            # per-core q/k/v width (NH * HS)
P = 128
NCORES = 8
DCH = D // P       # 8 contraction chunks for the projections
NW = S // 512      # 4 query windows of 512
NST = 4            # seq tiles (128 rows) per window
KCH = S // P       # 16 key chunks
ESHIFT = -4.0      # exp(0.25*s + ESHIFT); cancels in the softmax ratio

_cache = {}


def _build(has_bias=False):
    key = ("nc", has_bias)
    if key in _cache:
        return _cache[key]

    from contextlib import ExitStack

    import concourse.bass as bass
    import concourse.tile as tile
    from concourse import bacc, mybir
    from concourse.masks import make_identity, make_upper_triangular

    f32 = mybir.dt.float32
    f16 = mybir.dt.float16
    Exp = mybir.ActivationFunctionType.Exp
    Copy = mybir.ActivationFunctionType.Copy

    nc = bacc.Bacc(
        "TRN2", target_bir_lowering=False, debug=False, num_devices=NCORES
    )

    x_d = nc.dram_tensor("x", [S, D], f16, kind="ExternalInput").ap()
    wq_d = nc.dram_tensor("wq", [D, C], f16, kind="ExternalInput").ap()
    wk_d = nc.dram_tensor("wk", [D, C], f16, kind="ExternalInput").ap()
    wv_d = nc.dram_tensor("wv", [D, C], f16, kind="ExternalInput").ap()
    wp_d = nc.dram_tensor("wp", [C, D], f16, kind="ExternalInput").ap()
    # per-partition bias columns: col j<4 -> q col-tile j, col 4+j -> k col-tile j
    bqk_d = nc.dram_tensor("bqk", [P, 8], f32, kind="ExternalInput").ap()
    bv_d = nc.dram_tensor("bv", [1, C], f32, kind="ExternalInput").ap()
    bp_d = nc.dram_tensor("bp", [1, D], f32, kind="ExternalInput").ap()
    out_d = nc.dram_tensor("out", [S, D], f32, kind="ExternalOutput").ap()

    with tile.TileContext(nc) as tc, ExitStack() as ctx:
        ctx.enter_context(nc.allow_low_precision(reason="fp16 attention"))

        const = ctx.enter_context(tc.tile_pool(name="const", bufs=1))
        persist = ctx.enter_context(tc.tile_pool(name="persist", bufs=1))
        xload = ctx.enter_context(tc.tile_pool(name="xload", bufs=3))
        xtp = ctx.enter_context(tc.tile_pool(name="xtp", bufs=2))
        qtw = ctx.enter_context(tc.tile_pool(name="qtw", bufs=2))
        otw = ctx.enter_context(tc.tile_pool(name="otw", bufs=2))
        expool = ctx.enter_context(tc.tile_pool(name="expool", bufs=4))
        denpool = ctx.enter_context(tc.tile_pool(name="denpool", bufs=3))
        rhpool = ctx.enter_context(tc.tile_pool(name="rhpool", bufs=2))
        bcpool = ctx.enter_context(tc.tile_pool(name="bcpool", bufs=3))
        stpool = ctx.enter_context(tc.tile_pool(name="stpool", bufs=3))

        pS = ctx.enter_context(tc.tile_pool(name="pS", bufs=2, space="PSUM"))
        pPV = ctx.enter_context(tc.tile_pool(name="pPV", bufs=1, space="PSUM"))
        pDEN = ctx.enter_context(tc.tile_pool(name="pDEN", bufs=1, space="PSUM"))
        pMISC = ctx.enter_context(tc.tile_pool(name="pMISC", bufs=1, space="PSUM"))
        pAUX = ctx.enter_context(tc.tile_pool(name="pAUX", bufs=1, space="PSUM"))

        identf = const.tile([P, P], f32, tag="identf")
        make_identity(nc, identf)
        ident = const.tile([P, P], f16, tag="ident")
        nc.vector.tensor_copy(ident, identf)
        tri = const.tile([P, P], f32, tag="tri")
        make_upper_triangular(nc, tri, val=1.0, diag=True)  # tri[k,q]=1 iff q>=k
        eshift_sb = const.tile([P, 1], f32, tag="eshift")
        nc.vector.memset(eshift_sb, ESHIFT)
        ones8 = const.tile([P, 8], f32, tag="ones8")
        nc.vector.memset(ones8, 1.0)
        ohf = const.tile([1, 64], f32, tag="ohf")
        nc.vector.memset(ohf, 0.0)
        for h in range(NH):
            nc.vector.memset(ohf[0:1, h * 9:h * 9 + 1], 1.0)
        onehot = const.tile([1, NH, NH], f16, tag="onehot")
        nc.vector.tensor_copy(onehot, ohf[:].rearrange("p (a b) -> p a b", b=NH))
        bandf = const.tile([NH, 512], f32, tag="bandf")
        nc.gpsimd.memset(bandf, 1.0)
        nc.gpsimd.affine_select(
            out=bandf, in_=bandf, compare_op=mybir.AluOpType.is_ge,
            fill=0.0, base=0, pattern=[[1, 512]], channel_multiplier=-64)
        nc.gpsimd.affine_select(
            out=bandf, in_=bandf, compare_op=mybir.AluOpType.is_ge,
            fill=0.0, base=63, pattern=[[-1, 512]], channel_multiplier=64)
        sel = const.tile([NH, 512], f16, tag="sel")
        nc.vector.tensor_copy(sel, bandf[:])

        if has_bias:
            bqk_sb = const.tile([P, 8], f32, tag="bqk")
            nc.sync.dma_start(bqk_sb, bqk_d)
            bv_bc = const.tile([P, C], f32, tag="bv_bc")
            nc.sync.dma_start(
                bv_bc,
                bass.AP(tensor=bv_d.tensor, offset=bv_d.offset,
                        ap=[[0, P], list(bv_d.ap[-1])]),
            )
            bp_bc = const.tile([P, D], f32, tag="bp_bc")
            nc.sync.dma_start(
                bp_bc,
                bass.AP(tensor=bp_d.tensor, offset=bp_d.offset,
                        ap=[[0, P], list(bp_d.ap[-1])]),
            )

        wq_sb = persist.tile([P, DCH, C], f16, tag="wq")
        wk_sb = persist.tile([P, DCH, C], f16, tag="wk")
        wv_sb = persist.tile([P, DCH, C], f16, tag="wv")
        wp_sb = persist.tile([P, 4, D], f16, tag="wp")
        KT = persist.tile([P, 4, S], f16, tag="KT")
        Vt = persist.tile([P, KCH, NH * 65], f16, tag="Vt")

        # ---------- unit generators (emitted lazily for interleaving) ----------

        def xT_units(w, xT):
            """Load x rows of window w and transpose into xT [P, DCH, 512]."""
            for st in range(NST):
                def unit(w=w, st=st, xT=xT):
                    xt = xload.tile([P, D], f16, tag="xt")
                    row0 = (4 * w + st) * P
                    nc.sync.dma_start(xt, x_d[row0:row0 + P, :])
                    for dh in range(2):
                        ptr = pAUX.tile([P, 4, P], f16, tag="aux")
                        for j in range(4):
                            d = 4 * dh + j
                            nc.tensor.transpose(
                                ptr[:, j, :], xt[:, d * P:(d + 1) * P], ident[:])
                        nc.vector.tensor_copy(
                            xT[:, 4 * dh:4 * dh + 4, st * P:(st + 1) * P], ptr[:])
                yield unit

        def qk_units(w, xT, qt):
            """Q^T / K^T projections for window w from xT."""
            for ct in range(4):
                for qk in range(2):
                    def unit(w=w, ct=ct, qk=qk, xT=xT, qt=qt):
                        ps = pAUX.tile([P, 512], f32, tag="aux")
                        wsb = wq_sb if qk == 0 else wk_sb
                        for d in range(DCH):
                            nc.tensor.matmul(
                                ps, wsb[:, d, ct * P:(ct + 1) * P], xT[:, d, :],
                                start=(d == 0), stop=(d == DCH - 1))
                        dest = (qt[:, ct, :] if qk == 0
                                else KT[:, ct, w * 512:(w + 1) * 512])
                        if has_bias:
                            nc.vector.tensor_scalar_add(
                                dest, ps,
                                bqk_sb[:, qk * 4 + ct:qk * 4 + ct + 1])
                        else:
                            nc.vector.tensor_copy(dest, ps)
                    yield unit

        def v_units(w, xT):
            """V for the 4 key chunks of window w, head-grouped with ones col."""
            for st in range(NST):
                def unit(w=w, st=st, xT=xT):
                    kc = 4 * w + st
                    ps = pAUX.tile([P, 512], f32, tag="aux")
                    for d in range(DCH):
                        nc.tensor.matmul(ps, xT[:, d, st * P:(st + 1) * P],
                                         wv_sb[:, d, :],
                                         start=(d == 0), stop=(d == DCH - 1))
                    vtv = Vt[:, kc, :].rearrange("p (h c) -> p h c", c=65)
                    if has_bias:
                        nc.vector.tensor_add(
                            vtv[:, :, 0:64],
                            ps.rearrange("p (h c) -> p h c", c=64),
                            bv_bc[:].rearrange("p (h c) -> p h c", c=64))
                    else:
                        nc.vector.tensor_copy(
                            vtv[:, :, 0:64],
                            ps.rearrange("p (h c) -> p h c", c=64))
                    nc.vector.tensor_copy(vtv[:, :, 64:65],
                                          ones8[:, :].unsqueeze(2))
                yield unit

        def proj_units(w, ot):
            """Output projection for window w's 512 seq rows (partial over C)."""
            for st in range(NST):
                for cw in range(2):
                    def unit(w=w, st=st, cw=cw, ot=ot):
                        pp = pMISC.tile([P, 512], f32, tag="misc")
                        for j in range(4):
                            nc.tensor.matmul(
                                pp, ot[:, j, st * P:(st + 1) * P],
                                wp_sb[:, j, cw * 512:(cw + 1) * 512],
                                start=(j == 0), stop=(j == 3))
                        stg = stpool.tile([P, 512], f32, tag="stg")
                        if has_bias:
                            nc.vector.tensor_add(
                                stg, pp, bp_bc[:, cw * 512:(cw + 1) * 512])
                        else:
                            nc.vector.tensor_copy(stg, pp)
                        row0 = (4 * w + st) * P
                        nc.sync.dma_start(
                            out_d[row0:row0 + P, cw * 512:(cw + 1) * 512], stg)
                    yield unit

        def attention_head(w, h, qt, ot, den_ps):
            """Scores+exp+PV for one head; unnormalized O -> ot, denominator
            scattered into row h of den_ps via a K=1 one-hot matmul."""
            ct, po = h // 2, (h % 2) * 64
            pv = pPV.tile([65, 512], f32, tag="pv")
            last_kc = 4 * w + 3
            for g2 in range(2 * (w + 1)):      # 2-chunk half groups
                sc = pS.tile([P, 2, 512], f32, tag="sc")
                for rr in range(2):
                    kc = 2 * g2 + rr
                    nc.tensor.matmul(
                        sc[:, rr, :],
                        KT[po:po + 64, ct, kc * P:(kc + 1) * P],
                        qt[po:po + 64, ct, :],
                        start=True, stop=True)
                ex = expool.tile([P, 2, 512], f16, tag="ex")
                nc.scalar.activation(ex, sc, Exp, scale=0.25, bias=eshift_sb[:])
                for rr in range(2):
                    kc = 2 * g2 + rr
                    r = kc - 4 * w            # >=0 only inside the diag block
                    q0 = 0
                    if r >= 0:
                        # only q >= r*P can attend this chunk; clip the PV
                        # extent instead of zero-filling the masked region
                        q0 = r * P
                        nc.vector.tensor_mul(ex[:, rr, q0:q0 + P],
                                             ex[:, rr, q0:q0 + P], tri[:])
                    nc.tensor.matmul(pv[:, q0:], Vt[:, kc, h * 65:(h + 1) * 65],
                                     ex[:, rr, q0:],
                                     start=(kc == 0), stop=(kc == last_kc))
            # stash unnormalized O scaled by 1/4 (fp16 range headroom);
            # scatter the denominator into den_ps[h]
            nc.vector.tensor_scalar_mul(ot[po:po + 64, ct, :], pv[0:64, :], 0.25)
            den_h = denpool.tile([1, 512], f16, tag="den")
            nc.vector.tensor_copy(den_h, pv[64:65, :])
            nc.tensor.matmul(den_ps, onehot[:, h, :], den_h[:],
                             start=(h == 0), stop=(h == NH - 1))

        def norm_units(w, ot, den_ps):
            """Window-batched softmax normalization of ot (in place)."""
            shared = {}

            def recip_unit(ot=ot, den_ps=den_ps):
                rall = rhpool.tile([NH, 512], f32, tag="rall")
                nc.vector.reciprocal(rall, den_ps[:])
                rh = rhpool.tile([NH, 512], f16, tag="rh")
                # x4 so small reciprocals clear the fp16 subnormal floor; the
                # 4*(1/4) pair cancels in the final normalize multiply
                nc.vector.tensor_scalar_mul(rh, rall, 4.0)
                shared["rh"] = rh
            yield recip_unit
            for j in range(4):      # one unit per column-tile = 2 heads
                def unit(j=j, ot=ot):
                    rh = shared["rh"]
                    # sel[:, 128j:128j+128] row m selects head 2j + (m>=64)
                    bcp = pMISC.tile([P, 512], f32, tag="misc")
                    nc.tensor.matmul(bcp, sel[:, j * P:(j + 1) * P],
                                     rh[:], start=True, stop=True)
                    bc_sb = bcpool.tile([P, 512], f32, tag="bc")
                    nc.scalar.activation(bc_sb, bcp, Copy)
                    nc.vector.tensor_mul(ot[:, j, :], ot[:, j, :], bc_sb[:])
                yield unit

        # ---------------------------- schedule ----------------------------

        xT_cur = xtp.tile([P, DCH, 512], f16, tag="xT")
        qt_cur = qtw.tile([P, 4, 512], f16, tag="qt")
        xu0 = list(xT_units(0, xT_cur))
        xu0[0]()                      # x tile 0 in flight first: PE starts early
        for d in range(DCH):          # then stream the projection weights
            nc.sync.dma_start(wq_sb[:, d, :], wq_d[d * P:(d + 1) * P, :])
            nc.sync.dma_start(wk_sb[:, d, :], wk_d[d * P:(d + 1) * P, :])
        for u in xu0[1:]:
            u()
        for d in range(DCH):
            nc.sync.dma_start(wv_sb[:, d, :], wv_d[d * P:(d + 1) * P, :])
        for u in qk_units(0, xT_cur, qt_cur):
            u()
        for u in v_units(0, xT_cur):
            u()
        for j in range(4):            # proj weights are not needed until w1
            nc.sync.dma_start(wp_sb[:, j, :], wp_d[j * P:(j + 1) * P, :])

        pending = []            # units that must run after window w's heads
        for w in range(NW):
            ot_cur = otw.tile([P, 4, 512], f16, tag="ot")
            den_ps = pDEN.tile([NH, 512], f32, tag="den_ps")

            bg = list(pending)
            pending = []
            if w + 1 < NW:
                xT_next = xtp.tile([P, DCH, 512], f16, tag="xT")
                qt_next = qtw.tile([P, 4, 512], f16, tag="qt")
                bg += list(xT_units(w + 1, xT_next))
                bg += list(qk_units(w + 1, xT_next, qt_next))
                bg += list(v_units(w + 1, xT_next))
            else:
                xT_next = qt_next = None

            for h in range(NH):
                attention_head(w, h, qt_cur, ot_cur, den_ps)
                n_take = (len(bg) * (h + 1)) // NH - (len(bg) * h) // NH
                for _ in range(n_take):
                    bg.pop(0)()
            for u in bg:
                u()

            # reciprocal runs right away (den_ps is complete; frees its psum
            # before the next window's scatter); bc+proj interleave into the
            # next window's attention
            units = list(norm_units(w, ot_cur, den_ps))
            units[0]()
            pending = units[1:] + list(proj_units(w, ot_cur))

            xT_cur, qt_cur = xT_next, qt_next

        for u in pending:
            u()

    nc.compile()
    _cache[key] = nc
    return nc


def _make_in_maps(input_data, w_qkv, b_qkv, w_proj, b_proj):
    x = np.asarray(input_data, dtype=np.float32).astype(np.float16)
    wqkv = np.asarray(w_qkv, dtype=np.float32).astype(np.float16)
    bqkv = np.asarray(b_qkv, dtype=np.float32)
    wp = np.asarray(w_proj, dtype=np.float32).astype(np.float16)
    bp = np.asarray(b_proj, dtype=np.float32)

    in_maps = []
    for core in range(NCORES):
        b, g = core // 2, core % 2
        cs = slice(g * C, (g + 1) * C)
        bq = bqkv[0 * D:1 * D][cs]
        bk = bqkv[1 * D:2 * D][cs]
        bqk = np.empty((P, 8), np.float32)
        for j in range(4):
            bqk[:, j] = bq[j * P:(j + 1) * P]
            bqk[:, 4 + j] = bk[j * P:(j + 1) * P]
        in_maps.append({
            "x": np.ascontiguousarray(x[b]),
            "wq": np.ascontiguousarray(wqkv[:, 0 * D:1 * D][:, cs]),
            "wk": np.ascontiguousarray(wqkv[:, 1 * D:2 * D][:, cs]),
            "wv": np.ascontiguousarray(wqkv[:, 2 * D:3 * D][:, cs]),
            "wp": np.ascontiguousarray(wp[cs, :]),
            "bqk": bqk,
            "bv": np.ascontiguousarray(bqkv[2 * D:3 * D][cs]).reshape(1, C),
            "bp": (bp if g == 0 else np.zeros_like(bp)).reshape(1, D),
        })
    return in_maps


def kernel(input_data, w_qkv, b_qkv, w_proj, b_proj):
    from concourse.bass_utils import run_bass_kernel_spmd

    has_bias = bool(np.any(np.asarray(b_qkv)) or np.any(np.asarray(b_proj)))
    nc = _build(has_bias)
    in_maps = _make_in_maps(input_data, w_qkv, b_qkv, w_proj, b_proj)
    res = run_bass_kernel_spmd(nc, in_maps, core_ids=list(range(NCORES)))
    parts = [np.asarray(res.results[i]["out"]) for i in range(NCORES)]
    out = np.stack([parts[2 * b] + parts[2 * b + 1] for b in range(B)])
    return out.astype(np.float32)
